# revision 1
# baseline (speedup 1.0000x reference)
"""Trainium2 Bass kernel for the 8-qubit variational-circuit batch evaluator.

Math: the circuit state is a product state, the CNOT-ring "entangle" step is
linear over GF(2), and the output is a quadratic form in the state.  The whole
256-dim Kronecker product collapses analytically:

  ry=arctan(x)/2, rz=arctan(x^2)/2 per feature x, so
    cos(2ry)=1/sqrt(1+x^2),  sin(2ry)=x/sqrt(1+x^2)
    cos(2rz)=1/sqrt(1+x^4),  sin(2rz)=x^2/sqrt(1+x^4)

  With Z_q = 1+x_q^2, zz_q = 1+x_q^4, P27 = prod_{q=2..7} Z_q,
    A  = Z1 * P27
    BB = Z0*zz0*Z1*zz1
    out = C0 + C1/sqrt(A) + C2*x0*x1/sqrt(BB) + C3*x0*x1^3/sqrt(BB*P27)

  where (C0..C3) are scalars derived from the 3 complex rotation weights
  (computed on host: O(1) work).

Data-parallel: batch 131072 rows x 8 features sharded across 8 NeuronCores
(16384 rows each).  Layout on core: [128 partitions, R rows, 8 features].

Raw-Bass (manual semaphores): the TileContext teardown emits instructions this
walrus rejects, and ACT instructions only support one attached sync wait.
The rsqrt runs on the scalar engine as a raw InstActivation (bass's guard
against ActivationFunctionType.Rsqrt is bypassed; accuracy was validated on
hardware at <5e-5 rel over the full input domain [1, 3e10], and end-to-end
output error is ~1e-6).  square+reciprocal_sqrt live in one ACT table set
(reciprocal_sqrt_and_small), so there is no mid-kernel table switch; a dummy
activation at stream start prefetches the table during the input-DMA wait.

Pipelining: 2 chunks.  Input DMAs are issued concurrently from the otherwise
idle vector/gpsimd/tensor sequencers (their preambles finish before Sync's,
and parallel issue puts both chunks in flight ~0.7us earlier than serial
issue on one queue).  Chunk1's preprocessing (x^4, +1) runs on ACT to shorten
the DVE queue, which is the saturated engine; chunk0 keeps it on DVE so DVE
work starts as early as possible.

Scratch slot layout per row (44 f32 stride):
   0:8   x_q^2 (natural order)      20:25  [Z0Z1, Z2Z3, Z4Z5, Z6Z7, zzp]
   8:10  x0^4, x1^4                 25,27  Z2345, BB ; 26 P27 ; 28 A
  10:20  Z0..Z7, zz0, zz1           32:35  (K, R2, R1) = rsqrt(26:29)
  35 w ; 36:38 [x1^2 K, w R2] ; 38 f2 ; 39 f5 ; 40 f4
"""

import numpy as np

import concourse.bass as bass
from concourse import mybir
from concourse.bass_utils import run_bass_kernel_spmd

N_CORES = 8
BATCH = 131072
NQ = 8
B_LOCAL = BATCH // N_CORES  # 16384
P = 128
R_TOTAL = B_LOCAL // P      # 128 rows per partition
NCHUNK = 2
CHUNK_ROWS = [64, 64]
CHUNK_OFF = [0, 64]
NS = 41                     # scratch slots per row

F32 = mybir.dt.float32
AF = mybir.ActivationFunctionType
ALU = mybir.AluOpType


def _act_raw(nc, se, out, in_, func):
    """InstActivation without bass's Rsqrt accuracy guard (validated on HW)."""
    b = nc.const_aps.scalar_like(0.0, in_)
    ins = [se.lower_ap(in_), se.lower_ap(b),
           mybir.ImmediateValue(dtype=mybir.dt.float32, value=1.0),
           mybir.ImmediateValue(dtype=mybir.dt.float32, value=0.0)]
    return se.add_instruction(mybir.InstActivation(
        name=nc.get_next_instruction_name(), func=func,
        ins=ins, outs=[se.lower_ap(out)]))


def _chunk_part1(v, xt, s, squares_on_act):
    """DVE stage 1: products of the squares up to A (feeds the ACT rsqrt)."""
    if not squares_on_act:
        # x^2 -> s[0:8] (on DVE: starts right at data arrival, no ACT hop,
        # and DVE is ~1.3x faster per element than ACT here)
        v.tensor_mul(s[:, :, 0:8], xt[:, :, :], xt[:, :, :])
        # x0^4, x1^4
        v.tensor_mul(s[:, :, 8:10], s[:, :, 0:2], s[:, :, 0:2])
    # +1 -> s[10:20] = [Z0..Z7, zz0, zz1]
    v.tensor_scalar(s[:, :, 10:20], s[:, :, 0:10], 1.0, None, ALU.add)
    # pairwise -> s[20:25] = [Z0Z1, Z2Z3, Z4Z5, Z6Z7, zzp]
    v.tensor_mul(s[:, :, 20:25], s[:, :, 10:20:2], s[:, :, 11:20:2])
    # [Z2Z3, Z0Z1] * [Z4Z5, zzp] -> s25 = Z2345, s27 = BB
    v.tensor_mul(s[:, :, 25:29:2], s[:, :, 21:19:-1], s[:, :, 22:25:2])
    # P27 = Z2345 * Z6Z7 -> s26
    v.tensor_mul(s[:, :, 26:27], s[:, :, 25:26], s[:, :, 23:24])
    # w = x0*x1 -> s35 (independent; fills the pipe between dependent ops)
    v.tensor_mul(s[:, :, 35:36], xt[:, :, 0:1], xt[:, :, 1:2])
    # A = P27 * Z1 -> s28; rsqrt pack = s[26:29] = [P27, BB, A]
    return v.tensor_mul(s[:, :, 28:29], s[:, :, 26:27], s[:, :, 11:12])


def _chunk_part2(v, s, ct, ot):
    """DVE stage 2: after ACT rsqrt (s[32:35] = K, R2, R1) -> final combine."""
    # [x1^2*K, w*R2] -> s[36:38]
    v.tensor_mul(s[:, :, 36:38], s[:, :, 1:36:34], s[:, :, 32:34])
    # f2 = C3*(x1^2 K) + C2 -> s38
    v.tensor_scalar(s[:, :, 38:39], s[:, :, 36:37], ct[:, 3:4], ct[:, 2:3],
                    ALU.mult, ALU.add)
    # f5 = C1*R1 + C0 -> s39
    v.tensor_scalar(s[:, :, 39:40], s[:, :, 34:35], ct[:, 1:2], ct[:, 0:1],
                    ALU.mult, ALU.add)
    # f4 = (w R2) * f2 -> s40
    v.tensor_mul(s[:, :, 40:41], s[:, :, 37:38], s[:, :, 38:39])
    # out = f4 + f5
    return v.tensor_add(
        ot[:, :],
        s[:, :, 40:41].rearrange("p r one -> p (r one)"),
        s[:, :, 39:40].rearrange("p r one -> p (r one)"))


def _build_nc():
    nc = bass.Bass()
    x = nc.declare_dram_parameter("x", [B_LOCAL, NQ], F32, isOutput=False)
    co = nc.declare_dram_parameter("co", [4], F32, isOutput=False)
    y = nc.declare_dram_parameter("y", [B_LOCAL], F32, isOutput=True)

    xv = x.rearrange("(p r) q -> p r q", p=P)      # [128, 128, 8]
    yv = y.rearrange("(p r) -> p r", p=P)          # [128, 128]

    co_ap = co[:]
    co_bcast = bass.AP(tensor=co_ap.tensor, offset=co_ap.offset,
                       ap=[[0, P], [1, 4]])

    import contextlib
    with contextlib.ExitStack() as ctx:
        ct = ctx.enter_context(nc.sbuf_tensor("ct", [P, 4], F32))
        junk = ctx.enter_context(nc.sbuf_tensor("junk", [P, 2], F32))
        xts, ss, ots = [], [], []
        for c in range(NCHUNK):
            rc = CHUNK_ROWS[c]
            xts.append(ctx.enter_context(
                nc.sbuf_tensor(f"xt{c}", [P, rc, NQ], F32)))
            ss.append(ctx.enter_context(
                nc.sbuf_tensor(f"s{c}", [P, rc, NS], F32)))
            ots.append(ctx.enter_context(
                nc.sbuf_tensor(f"ot{c}", [P, rc], F32)))
        s_in0 = ctx.enter_context(nc.semaphore("s_in0"))
        s_in1 = ctx.enter_context(nc.semaphore("s_in1"))
        s_inct = ctx.enter_context(nc.semaphore("s_inct"))
        s_sq = ctx.enter_context(nc.semaphore("s_sq"))
        s_dve1 = ctx.enter_context(nc.semaphore("s_dve1"))
        s_rsq = ctx.enter_context(nc.semaphore("s_rsq"))
        s_dve2 = ctx.enter_context(nc.semaphore("s_dve2"))
        s_gps = ctx.enter_context(nc.semaphore("s_gps"))
        block = ctx.enter_context(nc.Block())

        @block.sync
        def _(sync):
            sync.dma_start(
                out=xts[0][:],
                in_=xv[:, CHUNK_OFF[0]:CHUNK_OFF[0] + CHUNK_ROWS[0], :]
            ).then_inc(s_in0, 16)
            sync.dma_start(
                out=xts[1][:],
                in_=xv[:, CHUNK_OFF[1]:CHUNK_OFF[1] + CHUNK_ROWS[1], :]
            ).then_inc(s_in1, 16)
            sync.dma_start(out=ct[:], in_=co_bcast).then_inc(s_inct, 16)
            for c in range(NCHUNK):
                sync.wait_ge(s_dve2, c + 1)
                sync.dma_start(
                    out=yv[:, CHUNK_OFF[c]:CHUNK_OFF[c] + CHUNK_ROWS[c]],
                    in_=ots[c][:]).then_inc(s_inct, 16)

        @block.scalar
        def _(scalar):
            # prefetch the ACT table set while the input DMA is in flight
            # (junk tile is uninitialized; the result is never read)
            _act_raw(nc, scalar, junk[:, 1:2], junk[:, 0:1], AF.Rsqrt)
            # chunk1 squares on ACT (overlap with DVE's chunk0 work);
            # chunk0's run on DVE, which starts earliest and is faster
            scalar.wait_ge(s_in1, 16)
            scalar.activation(ss[1][:, :, 0:8], xts[1][:, :, :],
                              AF.Square).then_inc(s_sq, 1)
            scalar.wait_ge(s_sq, 1)    # own-engine completion (RAW s[0:2])
            scalar.activation(ss[1][:, :, 8:10], ss[1][:, :, 0:2],
                              AF.Square).then_inc(s_sq, 1)
            for c in range(NCHUNK):
                scalar.wait_ge(s_dve1, c + 1)
                _act_raw(nc, scalar, ss[c][:, :, 32:35], ss[c][:, :, 26:29],
                         AF.Rsqrt).then_inc(s_rsq, 1)

        @block.vector
        def _(vector):
            vector.wait_ge(s_in0, 16)
            _chunk_part1(vector, xts[0], ss[0], False).then_inc(s_dve1, 1)
            vector.wait_ge(s_sq, 2)
            _chunk_part1(vector, xts[1], ss[1], True).then_inc(s_dve1, 1)
            vector.wait_ge(s_inct, 16)  # ct loaded
            for c in range(NCHUNK):
                vector.wait_ge(s_rsq, c + 1)
                _chunk_part2(vector, ss[c], ct, ots[c]).then_inc(s_dve2, 1)

    return nc


_NC = None


def _get_nc():
    global _NC
    if _NC is None:
        _NC = _build_nc()
    return _NC


def _host_coeffs(weights_re, weights_im):
    w = (np.asarray(weights_re, np.float64)
         + 1j * np.asarray(weights_im, np.float64)) * 0.5
    c, s = np.cos(w), np.sin(w)

    def rymat(i):
        return np.array([[c[i], -s[i]], [s[i], c[i]]])

    rot = rymat(2) @ (rymat(1) @ rymat(0))
    A, B = rot[0, 0], rot[0, 1]
    alpha = abs(B) ** 2
    beta = abs(A) ** 2 - abs(B) ** 2
    gam = A * np.conj(B)
    return np.array([alpha + beta / 2, beta / 2, gam.real, gam.imag],
                    dtype=np.float32)


def kernel(inputs, weights_re, weights_im):
    x = np.ascontiguousarray(np.asarray(inputs, dtype=np.float32))
    co = _host_coeffs(weights_re, weights_im)
    nc = _get_nc()
    shards = np.split(x, N_CORES, axis=0)
    in_maps = [{"x": sh, "co": co} for sh in shards]
    res = run_bass_kernel_spmd(nc, in_maps, list(range(N_CORES)))
    return np.concatenate([res.results[i]["y"] for i in range(N_CORES)])



# revision 4
# speedup vs baseline: 1.1096x; 1.1096x over previous
"""Trainium2 Bass kernel for the 8-qubit variational-circuit batch evaluator.

Math: the 256-dim product state collapses analytically (see derivation in the
v1 kernel).  With s_q = x_q^2, Z_q = 1+s_q, zz_q = 1+s_q^2:

  out = C0 + C1*rsqrt(A) + C2*w*rsqrt(BB) + C3*w*x1^2*rsqrt(BB*P27)

  P27 = prod_{q=2..7} Z_q,  A = Z1*P27,  BB = Z0*zz0*Z1*zz1,  w = x0*x1,
  C0..C3 host-derived scalars from the 3 complex rotation weights.

This version restructures the whole pipeline around rsqrt-EARLY form:
instead of multiplying Z_q products on DVE and taking 3 rsqrts, take
r_q = rsqrt(s_q + 1) for all 10 needed squares in ONE activation op (the
ACT engine computes func(in*scale + bias); bias=1.0 gives 1/sqrt(1+s)
directly), then multiply the small r-values:

  rsqrt(A) = K*r1,  rsqrt(BB) = (r0*rr0)*(r1*rr1) = R2,  K = r2*..*r7
  rsqrt(BB*P27) = K*R2

This deletes all ten "+1" adds, moves 10 elems/row to the otherwise-idle
ACT engine, and leaves DVE with ~25 elems/row (was 36).

Everything between the fp32 input and fp32 output runs in bf16 (harness
tolerance 2e-2; measured end-to-end rel err ~4e-4): DVE runs 2x/4x faster
on 16-bit operands when every non-scalar operand has a unit-stride
innermost dim >= 2.  Squares run AoS [P,R,10] (input is AoS from DRAM so
feature-minor keeps unit stride); the activation does the AoS->SoA
transpose for free (strided APs, same iteration order); the product tree
and fused scalar_tensor_tensor combines run SoA [P,slot,R].

The input is cast fp32->bf16 on host (round-to-nearest) before sharding,
halving input DMA bytes.  Coefficients C0..C3 are compile-time immediates
(the Bass program is built per weight-values and cached), removing the
coefficient DMA entirely.

Data-parallel: batch 131072 x 8 sharded across 8 NeuronCores (16384 rows
each), [128 partitions x 128 rows].  2 chunks of 64 rows pipeline
DMA-in -> squares(DVE) -> rsqrt(ACT) -> products(DVE) -> DMA-out, with
per-chunk DMAs issued from the Sync sequencer (chunk0's transfer fully
precedes chunk1's on the queue, so compute starts ~1.5us earlier than a
single bulk DMA).
"""

import numpy as np

import concourse.bass as bass
from concourse import mybir
from concourse.bass_utils import run_bass_kernel_spmd

N_CORES = 8
BATCH = 131072
NQ = 8
B_LOCAL = BATCH // N_CORES  # 16384
P = 128
R_TOTAL = B_LOCAL // P      # 128 rows per partition
CHUNK_ROWS = [64, 64]
CHUNK_OFF = [0, 64]
NCHUNK = len(CHUNK_ROWS)

F32 = mybir.dt.float32
BF16 = mybir.dt.bfloat16
AF = mybir.ActivationFunctionType
ALU = mybir.AluOpType


def _act_rsqrt_raw(nc, se, out, in_, bias):
    """InstActivation Rsqrt without bass's accuracy guard (validated on HW
    at <5e-5 rel over [1, 3e10]); bias is an AP so out = rsqrt(in + bias)."""
    ins = [se.lower_ap(in_), se.lower_ap(bias),
           mybir.ImmediateValue(dtype=mybir.dt.float32, value=1.0),
           mybir.ImmediateValue(dtype=mybir.dt.float32, value=0.0)]
    return se.add_instruction(mybir.InstActivation(
        name=nc.get_next_instruction_name(), func=AF.Rsqrt,
        ins=ins, outs=[se.lower_ap(out)]))


def _build_nc(coeffs):
    C0, C1, C2, C3 = [float(c) for c in coeffs]
    nc = bass.Bass()
    x = nc.declare_dram_parameter("x", [B_LOCAL, NQ], BF16, isOutput=False)
    y = nc.declare_dram_parameter("y", [B_LOCAL], F32, isOutput=True)

    xv = x.rearrange("(p r) q -> p r q", p=P)      # [128, 128, 8] bf16
    yv = y.rearrange("(p r) -> p r", p=P)          # [128, 128] f32

    import contextlib
    with contextlib.ExitStack() as ctx:
        junk = ctx.enter_context(nc.sbuf_tensor("junk", [P, 2], BF16))
        xts, sas, rts, pts, dts, wts, ots = [], [], [], [], [], [], []
        for c in range(NCHUNK):
            rc = CHUNK_ROWS[c]
            # AoS squares: [s2,s3,s4,s5,s6,s7, s0, s1, s0^2, s1^2]
            xts.append(ctx.enter_context(
                nc.sbuf_tensor(f"xt{c}", [P, rc, NQ], BF16)))
            sas.append(ctx.enter_context(
                nc.sbuf_tensor(f"sa{c}", [P, rc, 10], BF16)))
            # SoA rsqrt out: [r2,r3,r4,r5,r6,r7, r0, r1, rr0, rr1]
            rts.append(ctx.enter_context(
                nc.sbuf_tensor(f"rt{c}", [P, 10, rc], BF16)))
            # [p23, p45, p67, q0, q1]
            pts.append(ctx.enter_context(
                nc.sbuf_tensor(f"pt{c}", [P, 5, rc], BF16)))
            # [r2345, R2, K, R1, wR2, u, v, a]
            dts.append(ctx.enter_context(
                nc.sbuf_tensor(f"dt{c}", [P, 8, rc], BF16)))
            wts.append(ctx.enter_context(
                nc.sbuf_tensor(f"wt{c}", [P, 1, rc], BF16)))
            ots.append(ctx.enter_context(
                nc.sbuf_tensor(f"ot{c}", [P, 1, rc], F32)))
        s_in0 = ctx.enter_context(nc.semaphore("s_in0"))
        s_in1 = ctx.enter_context(nc.semaphore("s_in1"))
        s_sq = ctx.enter_context(nc.semaphore("s_sq"))
        s_rsq = ctx.enter_context(nc.semaphore("s_rsq"))
        s_out = ctx.enter_context(nc.semaphore("s_out"))
        s_done = ctx.enter_context(nc.semaphore("s_done"))
        s_ins = [s_in0, s_in1]
        block = ctx.enter_context(nc.Block())

        @block.sync
        def _(sync):
            for c in range(NCHUNK):
                sync.dma_start(
                    out=xts[c][:],
                    in_=xv[:, CHUNK_OFF[c]:CHUNK_OFF[c] + CHUNK_ROWS[c], :]
                ).then_inc(s_ins[c], 16)
            for c in range(NCHUNK):
                sync.wait_ge(s_out, c + 1)
                sync.dma_start(
                    out=yv[:, CHUNK_OFF[c]:CHUNK_OFF[c] + CHUNK_ROWS[c]],
                    in_=ots[c][:, 0, :]).then_inc(s_done, 16)

        @block.scalar
        def _(scalar):
            # prefetch the rsqrt ACT table during the input DMA wait
            bias1_junk = nc.const_aps.scalar_like(1.0, junk[:, 0:1])
            _act_rsqrt_raw(nc, scalar, junk[:, 1:2], junk[:, 0:1], bias1_junk)
            for c in range(NCHUNK):
                scalar.wait_ge(s_sq, c + 1)
                in_t = sas[c][:, :, :].rearrange("p r s -> p s r")
                bias1 = nc.const_aps.scalar_like(1.0, in_t)
                _act_rsqrt_raw(nc, scalar, rts[c][:, :, :], in_t,
                               bias1).then_inc(s_rsq, 1)

        @block.vector
        def _(vector):
            v = vector
            for c in range(NCHUNK):
                xt, sa, wt = xts[c], sas[c], wts[c]
                v.wait_ge(s_ins[c], 16)
                # squares (AoS, all-bf16 unit-stride => DVE fast mode)
                v.tensor_mul(sa[:, :, 0:6], xt[:, :, 2:8], xt[:, :, 2:8])
                v.tensor_mul(sa[:, :, 6:8], xt[:, :, 0:2], xt[:, :, 0:2])
                v.tensor_mul(sa[:, :, 8:10], sa[:, :, 6:8],
                             sa[:, :, 6:8]).then_inc(s_sq, 1)
                # w = x0*x1 (SoA dst; fp-independent of the rsqrt chain)
                v.tensor_mul(wt[:, 0, :], xt[:, :, 0], xt[:, :, 1])
            for c in range(NCHUNK):
                rt, pt, dt, wt, sa, ot = (rts[c], pts[c], dts[c], wts[c],
                                          sas[c], ots[c])
                v.wait_ge(s_rsq, c + 1)
                # [p23,p45,p67] ; [q0,q1]
                v.tensor_mul(pt[:, 0:3, :], rt[:, 0:6:2, :], rt[:, 1:6:2, :])
                v.tensor_mul(pt[:, 3:5, :], rt[:, 6:8, :], rt[:, 8:10, :])
                # [r2345, R2] = [p23,q0]*[p45,q1]
                v.tensor_mul(dt[:, 0:2, :], pt[:, 0:4:3, :], pt[:, 1:5:3, :])
                # K = r2345*p67 ; R1 = K*r1 ; wR2 = w*R2
                v.tensor_mul(dt[:, 2:3, :], dt[:, 0:1, :], pt[:, 2:3, :])
                v.tensor_mul(dt[:, 3:4, :], dt[:, 2:3, :], rt[:, 7:8, :])
                v.tensor_mul(dt[:, 4:5, :], wt[:, :, :], dt[:, 1:2, :])
                # u = (x1^2 * C3) * K   (x1^2 is AoS slot 7, transposed view)
                x1sq = sa[:, :, 7:8].rearrange("p r s -> p s r")
                v.scalar_tensor_tensor(dt[:, 5:6, :], x1sq, C3,
                                       dt[:, 2:3, :], ALU.mult, ALU.mult)
                # v = (u + C2) * wR2
                v.scalar_tensor_tensor(dt[:, 6:7, :], dt[:, 5:6, :], C2,
                                       dt[:, 4:5, :], ALU.add, ALU.mult)
                # a = (R1 * C1) + v
                v.scalar_tensor_tensor(dt[:, 7:8, :], dt[:, 3:4, :], C1,
                                       dt[:, 6:7, :], ALU.mult, ALU.add)
                # out = a + C0  (fp32 write)
                v.tensor_scalar(ot[:, 0:1, :], dt[:, 7:8, :], C0, None,
                                ALU.add).then_inc(s_out, 1)

    return nc


_NC_CACHE = {}


def _get_nc(coeffs):
    key = tuple(np.asarray(coeffs, np.float32).tolist())
    if key not in _NC_CACHE:
        _NC_CACHE[key] = _build_nc(key)
    return _NC_CACHE[key]


def _host_coeffs(weights_re, weights_im):
    w = (np.asarray(weights_re, np.float64)
         + 1j * np.asarray(weights_im, np.float64)) * 0.5
    c, s = np.cos(w), np.sin(w)

    def rymat(i):
        return np.array([[c[i], -s[i]], [s[i], c[i]]])

    rot = rymat(2) @ (rymat(1) @ rymat(0))
    A, B = rot[0, 0], rot[0, 1]
    alpha = abs(B) ** 2
    beta = abs(A) ** 2 - abs(B) ** 2
    gam = A * np.conj(B)
    return np.array([alpha + beta / 2, beta / 2, gam.real, gam.imag],
                    dtype=np.float32)


def _to_bf16(x):
    try:
        import ml_dtypes
        return x.astype(ml_dtypes.bfloat16)
    except ImportError:
        # round-to-nearest-even fp32 -> bf16 by hand, viewed as uint16
        u = x.view(np.uint32)
        rounded = (u + 0x7FFF + ((u >> 16) & 1)) >> 16
        return rounded.astype(np.uint16).view(np.dtype("uint16"))


def kernel(inputs, weights_re, weights_im):
    x = np.ascontiguousarray(np.asarray(inputs, dtype=np.float32))
    xb = _to_bf16(x)
    co = _host_coeffs(weights_re, weights_im)
    nc = _get_nc(co)
    shards = np.split(xb, N_CORES, axis=0)
    in_maps = [{"x": sh} for sh in shards]
    res = run_bass_kernel_spmd(nc, in_maps, list(range(N_CORES)))
    return np.concatenate([res.results[i]["y"] for i in range(N_CORES)])
